# revision 3
# baseline (speedup 1.0000x reference)
"""GAT layer kernel for 8 Trainium2 NeuronCores — fp16 gather edition.

Strategy (edge-parallel, node-partitioned output; zero collectives):
  - Shard edges across the 8 cores by dst-node range: core c owns nodes
    [c*N/8, (c+1)*N/8) and receives exactly the edges pointing into them.
  - Every core computes the full h = x @ W on device (PE) in fp16, writing
    a DRAM table of 128-fp16 rows [h(64) | s_src | s_dst | pad]; pairs of
    rows form 512B gather elements so indices fit in int16 for SWDGE.
  - Per-core, local nodes are renumbered by descending in-degree.  Edges
    scheduled in rank order form dense prefixes of the renumbered node
    space: the segment-sum becomes dense vector adds into an SBUF
    accumulator in fp16.
  - Gather calls are large (GQ tokens) and span rank boundaries; only the
    accumulate is split at boundaries.  Pad tokens point at a dedicated
    pad pair whose s_src = -60000 so exp() underflows to zero - no mask.
  - alpha normalization moves outside the segment sum; the reference's
    global max subtraction cancels algebraically and is skipped.
"""
import os
import sys
import numpy as np

_ABL = set(os.environ.get("KGAT_ABLATE", "").split(","))
_REP = int(os.environ.get("KGAT_REPEAT", "1"))
_GQ = int(os.environ.get("KGAT_GQ", "4096"))
_F32A = os.environ.get("KGAT_F32ACC", "0") == "1"
_NQ = int(os.environ.get("KGAT_NQ", "1"))

try:
    import concourse.bacc as bacc
except ImportError:
    sys.path.insert(0, "/opt/trn_rl_repo")
    import concourse.bacc as bacc
import concourse.tile as tile
import concourse.mybir as mybir
from concourse import bass_utils
from concourse.masks import make_identity

C = 8                 # cores
TROW = 128            # fp16 per table row:  [h(64) | s_src | s_dst | pad]
PAIR = 2 * TROW       # fp16 per gather element (512B)
PAD_S = -60000.0      # pad-row s_src: exp(0.2*PAD_S) == 0

F32 = mybir.dt.float32
F16 = mybir.dt.float16
I16 = mybir.dt.int16


def _wrap16(a):
    """[K] int16 -> [128, K//16]: token j at [j%16, j//16], replicated to
    the 8 gpsimd core groups."""
    w = np.ascontiguousarray(a.reshape(-1, 16).T)
    return np.tile(w, (8, 1))


def _tl(a):
    """[K] -> [128, K//128] tile layout: token j at [j%128, j//128]."""
    return np.ascontiguousarray(a.reshape(-1, 128).T)


def _nxp(N):
    """Nodes padded so phase B runs in full 1024-node batches of 256-chunks."""
    return -(-N // 1024) * 1024


def _prep_x(x):
    """Host: pad x to NXP nodes, cast fp16, and interleave columns per
    256-node chunk (evens then odds) so phase B's table write pairs
    consecutive DRAM rows into 512B descriptors."""
    x = np.asarray(x, dtype=np.float32)
    N, F = x.shape
    NXP = _nxp(N)
    xp = np.zeros((F, NXP), np.float16)
    xp[:, :N] = x.T.astype(np.float16)
    v = xp.reshape(F, NXP // 256, 128, 2)
    return np.ascontiguousarray(
        v.transpose(0, 1, 3, 2).reshape(F, NXP))


def _prep(edge_index, edge_weight, N):
    """Shard + schedule (index bookkeeping only).

    Returns (ranks, calls, per_core_inputs, perms, tot, NLP) where
      ranks = [(o128, nb)]           block range of each rank
      calls = [(K, [(s0, ns, boff)])] gather calls with accumulate runs
    """
    NL = N // C
    PADP = _nxp(N) // 2             # pad pair index (rows NXP, NXP+1)
    src = np.asarray(edge_index[0], dtype=np.int64)
    dst = np.asarray(edge_index[1], dtype=np.int64)
    w = np.asarray(edge_weight, dtype=np.float32)

    cores = []
    max_cnt = np.zeros(0, np.int64)
    for c in range(C):
        m = (dst >= c * NL) & (dst < (c + 1) * NL)
        s_c = src[m]
        d_c = dst[m] - c * NL
        w_c = w[m]
        deg = np.bincount(d_c, minlength=NL)
        perm = np.argsort(-deg, kind="stable")          # position -> natural
        order = np.argsort(d_c, kind="stable")          # edges grouped by dst
        starts = np.zeros(NL + 1, np.int64)
        starts[1:] = np.cumsum(deg)
        maxdeg = int(deg.max()) if deg.size else 0
        hist = np.bincount(deg, minlength=maxdeg + 2)
        cnt = NL - np.cumsum(hist)[:maxdeg + 1]          # #nodes with deg > r
        cnt = cnt[cnt > 0]
        cores.append(dict(s=s_c, w=w_c, perm=perm, order=order,
                          starts=starts, cnt=cnt))
        if len(cnt) > len(max_cnt):
            mc = np.zeros(len(cnt), np.int64)
            mc[:len(max_cnt)] = max_cnt
            max_cnt = mc
        max_cnt[:len(cnt)] = np.maximum(max_cnt[:len(cnt)], cnt)

    # shared flat schedule: rank r occupies blocks [o128_r, o128_r + nb_r)
    ranks = []
    rank_base = []
    tot = 0
    for r in range(len(max_cnt)):
        c128 = int(-(-max_cnt[r] // 128) * 128)
        rank_base.append(tot)
        ranks.append((tot // 128, c128 // 128))
        tot += c128

    # gather calls of up to _GQ tokens; accumulate runs split at rank edges
    bounds = sorted(rb // 128 for rb in rank_base)      # block idx of starts
    calls = []
    p = 0
    while p < tot:
        K = min(_GQ, tot - p)
        gb0 = p // 128
        sl = K // 128
        runs = []
        s0 = 0
        while s0 < sl:
            gb = gb0 + s0
            # rank containing gb
            import bisect
            ri = bisect.bisect_right(bounds, gb) - 1
            rank_end = bounds[ri + 1] if ri + 1 < len(bounds) else tot // 128
            ns = min(sl - s0, rank_end - gb)
            boff = gb - bounds[ri]
            runs.append((s0, ns, boff))
            s0 += ns
        calls.append((K, tuple(runs)))
        p += K

    NLP = -(-NL // 128) * 128     # padded node positions per core
    per_core = []
    for c in range(C):
        cc = cores[c]
        perm, order, starts, cnt = cc["perm"], cc["order"], cc["starts"], cc["cnt"]
        pair = np.full(tot, PADP, np.int16)
        par = np.zeros(tot, np.float16)
        wt = np.ones(tot, np.float16)
        for r in range(len(max_cnt)):
            n = int(cnt[r]) if r < len(cnt) else 0
            if n == 0:
                continue
            o = rank_base[r]
            eid = order[starts[perm[:n]] + r]
            sg = cc["s"][eid]
            pair[o:o + n] = (sg >> 1).astype(np.int16)
            par[o:o + n] = (sg & 1).astype(np.float16)
            wt[o:o + n] = cc["w"][eid].astype(np.float16)

        sidx = _wrap16(pair)
        parT = _tl(par)
        wgtT = _tl(wt)

        gnode = np.zeros(NLP, np.int64)
        gnode[:NL] = c * NL + perm
        pgidx = _wrap16((gnode >> 1).astype(np.int16))
        ppar = _tl((gnode & 1).astype(np.float16))

        per_core.append(dict(sidx=sidx, par=parT, wgt=wgtT,
                             pgidx=pgidx, ppar=ppar))

    return tuple(ranks), tuple(calls), per_core, \
        [cores[c]["perm"] for c in range(C)], tot, NLP


_BUILD_CACHE = {}


def _build(N, F, O, ranks, calls, tot, NLP):
    key = (N, F, O, ranks, calls, tot, NLP, _REP, _GQ, _F32A,
           tuple(sorted(_ABL)))
    if key in _BUILD_CACHE:
        return _BUILD_CACHE[key]
    NB = NLP // 128           # node blocks per core
    TOT16 = tot // 16
    TOT128 = tot // 128
    NXP = _nxp(N)             # padded nodes (full 1024-node batches)
    NT = NXP + 128            # table rows incl pad pair
    GSL = _GQ // 128          # max slots per gather call
    ACC = F32 if _F32A else F16

    nc = bacc.Bacc("TRN2", target_bir_lowering=False,
                   dynamic_dma_scratch_size=32768, num_swdge_queues=_NQ)
    x_t = nc.dram_tensor("x", [F, NXP], F16, kind="ExternalInput")
    w_t = nc.dram_tensor("W", [F, O], F32, kind="ExternalInput")
    a_t = nc.dram_tensor("a", [2 * O], F32, kind="ExternalInput")
    sidx_t = nc.dram_tensor("sidx", [128, TOT16], I16, kind="ExternalInput")
    par_t = nc.dram_tensor("par", [128, TOT128], F16, kind="ExternalInput")
    wgt_t = nc.dram_tensor("wgt", [128, TOT128], F16, kind="ExternalInput")
    pgidx_t = nc.dram_tensor("pgidx", [128, NLP // 16], I16,
                             kind="ExternalInput")
    ppar_t = nc.dram_tensor("ppar", [128, NB], F16, kind="ExternalInput")
    out_t = nc.dram_tensor("out", [128, NB * O], F32, kind="ExternalOutput")

    with tile.TileContext(nc) as tc:
        with (
            tc.tile_pool(name="persist", bufs=1) as pp,
            tc.tile_pool(name="dram", bufs=1, space="DRAM") as dp,
            tc.tile_pool(name="psum", bufs=2, space="PSUM") as psp,
            tc.tile_pool(name="work", bufs=4) as wp,
            tc.tile_pool(name="gpool", bufs=3) as gp,
        ):
            table = dp.tile([NT, TROW], F16)
            tpair = table[:].rearrange("(p two) r -> p (two r)", two=2)

            # ---- phase A: W_aug = [W | W@a1 | W@a2] in fp16 ----
            ident = pp.tile([128, 128], F32)
            make_identity(nc, ident[:])
            ws = pp.tile([128, O], F32)
            nc.sync.dma_start(ws[:], w_t[:])
            a1 = pp.tile([O, 1], F32)
            a2 = pp.tile([O, 1], F32)
            nc.sync.dma_start(a1[:], a_t[:O, None])
            nc.sync.dma_start(a2[:], a_t[O:, None])
            wtp = psp.tile([O, 128], F32, space="PSUM")
            nc.tensor.transpose(out=wtp[:], in_=ws[:], identity=ident[:])
            wts = pp.tile([O, 128], F32)
            nc.vector.tensor_copy(wts[:], wtp[:])
            vab = psp.tile([128, 2], F32, space="PSUM")
            nc.tensor.matmul(out=vab[:, 0:1], lhsT=wts[:], rhs=a1[:],
                             start=True, stop=True)
            nc.tensor.matmul(out=vab[:, 1:2], lhsT=wts[:], rhs=a2[:],
                             start=True, stop=True)
            waug = pp.tile([128, O + 2], F16)
            nc.vector.tensor_copy(waug[:, :O], ws[:])
            nc.vector.tensor_copy(waug[:, O:], vab[:])

            for _rep in range(_REP):
              # ---- phase B: table rows [h | s_src | s_dst | pad] ----
              # x columns are host-interleaved so matmul group 2j+t holds
              # the (t ? odd : even) nodes of 256-chunk j; the paired write
              # below lands 512B-contiguous row pairs in DRAM.
              XB = 8
              nbt = NXP // (XB * 128)
              for t in range(0 if "phaseb" in _ABL else nbt):
                  n0 = t * XB * 128
                  xts = wp.tile([F, XB * 128], F16, tag="xts")
                  nc.sync.dma_start(xts[:], x_t[:, n0:n0 + XB * 128])
                  hs = wp.tile([128, XB, TROW], F16, tag="hs")
                  for g in range(XB):
                      hp = psp.tile([128, O + 2], F32, space="PSUM", tag="hp")
                      nc.tensor.matmul(
                          out=hp[:],
                          lhsT=xts[:, g * 128:(g + 1) * 128],
                          rhs=waug[:], start=True, stop=True)
                      nc.vector.tensor_copy(hs[:, g, :O + 2], hp[:])
                      nc.vector.memset(hs[:, g, O + 2:], 0.0)
                  nc.scalar.dma_start(
                      table[n0:n0 + XB * 128, :].rearrange(
                          "(j p two) r -> p j (two r)", p=128, two=2),
                      hs[:].rearrange("p (j two) r -> p j (two r)", two=2))
              # pad pair rows NXP, NXP+1: h = 0, s_src = s_dst = PAD_S
              ptile = wp.tile([2, TROW], F16, tag="ptile")
              nc.vector.memset(ptile[:], 0.0)
              nc.vector.memset(ptile[:, O:O + 2], PAD_S)
              nc.scalar.dma_start(table[NXP:NXP + 2, :], ptile[:])

              # ---- stream preload ----
              sidxS = pp.tile([128, TOT16], I16)
              nc.scalar.dma_start(sidxS[:], sidx_t[:])
              parS = pp.tile([128, TOT128], F16)
              nc.scalar.dma_start(parS[:], par_t[:])
              wgtS = pp.tile([128, TOT128], F16)
              nc.scalar.dma_start(wgtS[:], wgt_t[:])
              pgS = pp.tile([128, NLP // 16], I16)
              nc.scalar.dma_start(pgS[:], pgidx_t[:])
              pparS = pp.tile([128, NB], F16)
              nc.scalar.dma_start(pparS[:], ppar_t[:])

              # ---- phase B2: s_dst in permuted order, expanded per rank ----
              sdp = pp.tile([128, NB], F16)
              p = 0
              while p < NLP:
                  K = min(_GQ, NLP - p)
                  sl = K // 128
                  b0 = p // 128
                  pg = gp.tile([128, GSL, PAIR], F16, tag="gt")
                  nc.gpsimd.dma_gather(
                      out_ap=pg[:, :sl, :], in_ap=tpair,
                      idxs_ap=pgS[:, p // 16:(p + K) // 16],
                      num_idxs=K, num_idxs_reg=K, elem_size=PAIR,
                      queue_num=0, single_packet=False)
                  lo = pg[:, :sl, O + 1]
                  hi = pg[:, :sl, TROW + O + 1]
                  tmp = wp.tile([128, GSL], F16, tag="sdtmp")
                  nc.vector.tensor_sub(tmp[:, :sl], hi, lo)
                  nc.vector.tensor_mul(tmp[:, :sl], tmp[:, :sl],
                                       pparS[:, b0:b0 + sl])
                  nc.vector.tensor_add(sdp[:, b0:b0 + sl], lo, tmp[:, :sl])
                  p += K
              sdpS = pp.tile([128, TOT128], F16)
              for (o128, nb) in ranks:
                  nc.vector.tensor_copy(sdpS[:, o128:o128 + nb],
                                        sdp[:, :nb])

              # ---- edge phase: dense rank accumulation ----
              accum = pp.tile([128, NB, O + 1], ACC)
              nc.vector.memset(accum[:], 0.0)
              o16 = 0
              gb0 = 0
              qn = 0
              for (K, runs) in calls:
                  sl = K // 128
                  if "gather" in _ABL:
                      o16 += K // 16
                      gb0 += sl
                      continue
                  gt = gp.tile([128, GSL, PAIR], F16, tag="gt")
                  nc.gpsimd.dma_gather(
                      out_ap=gt[:, :sl, :], in_ap=tpair,
                      idxs_ap=sidxS[:, o16:o16 + K // 16],
                      num_idxs=K, num_idxs_reg=K, elem_size=PAIR,
                      queue_num=qn, single_packet=False)
                  qn = (qn + 1) % _NQ
                  o16 += K // 16
                  if "dve" in _ABL:
                      gb0 += sl
                      continue
                  par = parS[:, gb0:gb0 + sl]
                  wgt = wgtS[:, gb0:gb0 + sl]
                  sel = wp.tile([128, GSL, O + 1], F16, tag="sel")
                  parb = par[:, :, None].to_broadcast([128, sl, O + 1])
                  nc.vector.tensor_sub(sel[:, :sl, :],
                                       gt[:, :sl, TROW:TROW + O + 1],
                                       gt[:, :sl, :O + 1])
                  nc.vector.tensor_mul(sel[:, :sl, :], sel[:, :sl, :], parb)
                  nc.vector.tensor_add(sel[:, :sl, :], sel[:, :sl, :],
                                       gt[:, :sl, :O + 1])

                  # e = s_src + s_dst ; alpha = exp(leaky(e) * w)
                  e = wp.tile([128, GSL], F16, tag="e")
                  nc.vector.tensor_add(e[:, :sl], sel[:, :sl, O],
                                       sdpS[:, gb0:gb0 + sl])
                  lk = wp.tile([128, GSL], F16, tag="lk")
                  nc.scalar.activation(lk[:, :sl], e[:, :sl],
                                       mybir.ActivationFunctionType.Prelu,
                                       alpha=0.2)
                  nc.vector.tensor_mul(lk[:, :sl], lk[:, :sl], wgt)
                  ax = wp.tile([128, GSL], F16, tag="ax")
                  nc.scalar.activation(ax[:, :sl], lk[:, :sl],
                                       mybir.ActivationFunctionType.Exp)

                  # msg = [h*axp | axp]; sel col O set to 1 first
                  nc.vector.memset(sel[:, :sl, O], 1.0)
                  nc.vector.tensor_mul(
                      sel[:, :sl, :],
                      sel[:, :sl, :],
                      ax[:, :sl, None].to_broadcast([128, sl, O + 1]))
                  for (s0, ns, boff) in runs:
                      nc.vector.tensor_add(accum[:, boff:boff + ns, :],
                                           accum[:, boff:boff + ns, :],
                                           sel[:, s0:s0 + ns, :])
                  gb0 += sl

              # ---- final: out = elu(S / (alpha_sum + 1e-8)) ----
              acf = pp.tile([128, NB, O + 1], F32)
              nc.vector.tensor_copy(acf[:], accum[:])
              rc = pp.tile([128, NB], F32)
              nc.vector.tensor_scalar(out=rc[:], in0=acf[:, :, O],
                                      scalar1=1e-8, scalar2=None,
                                      op0=mybir.AluOpType.add)
              nc.vector.reciprocal(rc[:], rc[:])
              ov = pp.tile([128, NB, O], F32)
              nc.vector.tensor_mul(ov[:], acf[:, :, :O],
                                   rc[:, :, None].to_broadcast([128, NB, O]))
              neg = pp.tile([128, NB, O], F32)
              nc.vector.tensor_scalar(out=neg[:], in0=ov[:], scalar1=0.0,
                                      scalar2=None, op0=mybir.AluOpType.min)
              nc.scalar.activation(neg[:], neg[:],
                                   mybir.ActivationFunctionType.Exp)
              nc.vector.tensor_scalar(out=ov[:], in0=ov[:], scalar1=0.0,
                                      scalar2=-1.0, op0=mybir.AluOpType.max,
                                      op1=mybir.AluOpType.add)
              nc.vector.tensor_add(ov[:], ov[:], neg[:])
              nc.scalar.dma_start(
                  out_t[:].rearrange("p (b f) -> p b f", b=NB), ov[:])

    nc.compile()
    _BUILD_CACHE[key] = nc
    return nc


def kernel(x, edge_index, edge_weight, W, a):
    x = np.asarray(x, dtype=np.float32)
    xT = _prep_x(x)
    W = np.ascontiguousarray(np.asarray(W, dtype=np.float32))
    a = np.ascontiguousarray(np.asarray(a, dtype=np.float32))
    N, F = x.shape
    O = W.shape[1]
    NL = N // C

    ranks, calls, per_core, perms, tot, NLP = _prep(edge_index, edge_weight, N)
    nc = _build(N, F, O, ranks, calls, tot, NLP)

    in_maps = []
    for c in range(C):
        pc = per_core[c]
        in_maps.append({
            "x": xT, "W": W, "a": a,
            "sidx": pc["sidx"], "par": pc["par"], "wgt": pc["wgt"],
            "pgidx": pc["pgidx"], "ppar": pc["ppar"],
        })
    res = bass_utils.run_bass_kernel_spmd(nc, in_maps, core_ids=list(range(C)))

    NB = NLP // 128
    out = np.empty((N, O), np.float32)
    for c in range(C):
        op = res.results[c]["out"].reshape(128, NB, O)
        op = np.ascontiguousarray(op.transpose(1, 0, 2)).reshape(NLP, O)
        out[c * NL + perms[c]] = op[:NL]
    return out
